# revision 1
# baseline (speedup 1.0000x reference)
"""Trainium2 Bass kernel for nn_AttentionModel (S=2048, B=32, H=1024).

Math: reference computes
    energy[b,s] = (enc[s,b,:] @ We.T + (h @ Wh.T + bias)) @ v  ; out = softmax_s(energy)
Since softmax is shift-invariant and the (h @ Wh.T + bias) @ v term is constant
over s, the output reduces exactly to
    out[b, 0, s] = softmax_s( enc[s,b,:] . u ),   u = v[0] @ We   (We = attn_W[:, H:])
So the kernel is a memory-bound [S*B, H] x [H] matvec + row softmax.

Sharding: data-parallel over batch B across 8 cores (4 batches/core).
Device layout per core: enc [BL, H, S] (h on SBUF partitions, s on free dim),
PE matmul contracts h in chunks of 128 (lhsT = u chunk [128,1], rhs = enc tile
[128,512], fp32r, PSUM-accumulated), softmax on ScalarE/VectorE.
"""

import numpy as np

import concourse.bass as bass
import concourse.tile as tile
from concourse import bacc, mybir
from concourse.bass_utils import run_bass_kernel_spmd

S, B, H = 2048, 32, 1024
NCORES = 8
BL = B // NCORES  # batches per core
MM_N = 512        # matmul moving free dim (fp32 max, 1 PSUM bank)


def build_nc(bl=BL, h=H, s=S, enc_bufs=4, jpd=4, use_f32r=True, debug=False,
             taper=True):
    """Build the per-core Bass program (SPMD: same program, different data)."""
    nc = bacc.Bacc()
    f32 = mybir.dt.float32
    jc = h // 128      # h chunks (contraction tiles)
    ns = s // MM_N     # matmul slices per output row
    jpd = min(jpd, jc) # h-chunks per DMA
    nd = jc // jpd     # DMAs per batch
    # Per-batch DMA chunking (in h-chunks of 128). Large chunks sustain the
    # best HBM rate; the last batch tapers so the cold-PE tail after the
    # final chunk is only a few matmuls.
    # Last batch: small chunks first, then one big 4-h-chunk block streamed as
    # ns per-slice sub-DMAs. Keeps the DMA queue DEEP at the end of the
    # stream (shallow tail queues get starved by the HBM-pair partner under
    # contention) while still letting each slice finish as its bytes land.
    plan = [[jpd] * nd for _ in range(bl)]
    split_last = taper and jc == 8 and jpd in (4, 8)
    if split_last:
        plan[bl - 1] = [1, 1, 2, 4]

    mm_dt = mybir.dt.float32r if use_f32r else f32
    enc_d = nc.declare_dram_parameter("enc", [bl, h, s], mm_dt, isOutput=False)
    u_d = nc.declare_dram_parameter("u", [128, jc], mm_dt, isOutput=False)
    # Flash-softmax split: out rows are exp(e_slice - max_slice) per 512-wide
    # slice; per-slice sums and (negated) maxes are returned and the host
    # does the 4-element rescale + divide during the gather (the heavy work -
    # energy matvec, maxes, exp, sums - all happens on device).
    out_d = nc.declare_dram_parameter("out", [bl, s], f32, isOutput=True)
    sums_d = nc.declare_dram_parameter("sums", [bl, ns], f32, isOutput=True)
    maxs_d = nc.declare_dram_parameter("maxs", [bl, ns], f32, isOutput=True)
    if debug:
        dbg_e = nc.declare_dram_parameter("dbg_e", [bl, s], f32, isOutput=True)
        dbg_p = nc.declare_dram_parameter("dbg_p", [bl, s], f32, isOutput=True)
        dbg_m = nc.declare_dram_parameter("dbg_m", [bl, 4], f32, isOutput=True)

    # Bacc's compile() legalizes multi-semaphore waits (splitting them into
    # EventSemaphore chains), so the structure below can stay simple. Big enc
    # loads go via HWDGE (nc.sync); small transfers via SWDGE (nc.gpsimd).
    with tile.TileContext(nc) as tc:
        with (
            tc.tile_pool(name="up", bufs=1) as up,
            tc.tile_pool(name="encp", bufs=enc_bufs) as encp,
            tc.tile_pool(name="smp", bufs=2) as smp,
            tc.tile_pool(name="op", bufs=1) as op,
            tc.tile_pool(name="psp", bufs=2, space="PSUM") as psp,
        ):
            # Issue the first enc load before anything else so the DMA
            # pipeline starts immediately; the tiny u load follows it.
            t0 = encp.tile([128, plan[0][0], s], mm_dt, name="t",
                           padded_shape=[128, jpd, s])
            nc.sync.dma_start(
                t0[:],
                enc_d[0, 0:plan[0][0] * 128, :].rearrange("(j p) s -> p j s", p=128),
            )
            u_sb = up.tile([128, jc], mm_dt)
            nc.sync.dma_start(u_sb[:], u_d[:])

            o_sum = op.tile([1, bl, ns], f32)
            o_max = op.tile([1, bl, ns], f32)
            for b in range(bl):
                # Accumulate this batch's energy row in PSUM [1, s] (4 banks,
                # partition 0); 8 fp32r matmuls per 512-wide slice.
                e_ps = psp.tile([1, s], f32)
                m4 = smp.tile([1, ns], f32)
                s4 = smp.tile([1, ns], f32)
                p_exp = smp.tile([1, s], f32)
                last = b == bl - 1 and split_last
                j = 0
                for d, cw in enumerate(plan[b]):
                    split = ns if (last and d == len(plan[b]) - 1) else 1
                    for sub in range(split):
                        # For the final block, stream each 512-wide s-slice
                        # as its own DMAs - and split off the very last
                        # h-chunk (the slice's finishing matmul) into its own
                        # small DMA, so only ONE matmul + max + exp sit after
                        # the slice's last bytes.
                        if b == 0 and d == 0:
                            t = t0
                        elif split == 1:
                            scols = s
                            t = encp.tile([128, cw, scols], mm_dt, name="t",
                                          padded_shape=[128, jpd, s])
                            src = enc_d[b, j * 128:(j + cw) * 128, :]
                            nc.sync.dma_start(
                                t[:], src.rearrange("(j p) s -> p j s", p=128)
                            )
                        else:
                            scols = s // split
                            t = encp.tile([128, cw, scols], mm_dt, name="t",
                                          padded_shape=[128, jpd, s])
                            sc = slice(sub * scols, (sub + 1) * scols)
                            src_a = enc_d[b, j * 128:(j + cw - 1) * 128, sc]
                            nc.sync.dma_start(
                                t[:, 0:cw - 1, :],
                                src_a.rearrange("(j p) s -> p j s", p=128),
                            )
                            src_b = enc_d[b, (j + cw - 1) * 128:(j + cw) * 128, sc]
                            nc.sync.dma_start(
                                t[:, cw - 1:cw, :],
                                src_b.rearrange("(j p) s -> p j s", p=128),
                            )
                        for jl in range(cw):
                            sss = range(ns) if split == 1 else [sub]
                            for ss in sss:
                                coff = 0 if split == 1 else -ss * MM_N
                                nc.tensor.matmul(
                                    e_ps[:, ss * MM_N:(ss + 1) * MM_N],
                                    u_sb[:, j + jl:j + jl + 1],
                                    t[:, jl, ss * MM_N + coff:
                                       (ss + 1) * MM_N + coff],
                                    start=(j + jl == 0),
                                    stop=(j + jl == jc - 1),
                                )
                                if j + jl == jc - 1:
                                    # This slice's group is complete: negated
                                    # max, then exp with fused slice-sum,
                                    # overlapping remaining matmuls/DMAs.
                                    nc.vector.reduce_max(
                                        m4[:, ss:ss + 1],
                                        e_ps[:, ss * MM_N:(ss + 1) * MM_N],
                                        axis=mybir.AxisListType.X,
                                        negate=True,
                                    )
                                    nc.scalar.activation(
                                        p_exp[:, ss * MM_N:(ss + 1) * MM_N],
                                        e_ps[:, ss * MM_N:(ss + 1) * MM_N],
                                        mybir.ActivationFunctionType.Exp,
                                        bias=m4[:, ss:ss + 1],
                                        accum_out=s4[:, ss:ss + 1],
                                    )
                    j += cw
                nc.gpsimd.dma_start(out_d[b:b + 1, :], p_exp[:])
                nc.vector.tensor_copy(o_sum[:, b, :], s4[:])
                nc.vector.tensor_copy(o_max[:, b, :], m4[:])
            # Keep the partition dim explicit on the SBUF side: x[0] would
            # make the free dim `bl` look like a partition dim.
            nc.gpsimd.dma_start(sums_d[:], o_sum[0:1, :, :])
            nc.gpsimd.dma_start(maxs_d[:], o_max[0:1, :, :])
    nc.compile()
    return nc


def _prep_inputs(encoder_outputs, attn_W, v):
    encoder_outputs = np.asarray(encoder_outputs, dtype=np.float32)
    attn_W = np.asarray(attn_W, dtype=np.float32)
    v = np.asarray(v, dtype=np.float32)
    h = attn_W.shape[0]
    # u = v[0] @ We in float64 (host-side, tiny)
    u = (v[0].astype(np.float64) @ attn_W[:, h:].astype(np.float64)).astype(np.float32)
    u128 = np.ascontiguousarray(u.reshape(h // 128, 128).T)  # [128, jc]
    in_maps = []
    for c in range(NCORES):
        sl = encoder_outputs[:, c * BL:(c + 1) * BL, :]
        enc_c = np.ascontiguousarray(sl.transpose(1, 2, 0))  # [BL, H, S]
        in_maps.append({"enc": enc_c, "u": u128})
    return in_maps


def run(encoder_outputs, rnn_hidden, attn_W, attn_b, v, trace=False, **bass_kwargs):
    in_maps = _prep_inputs(encoder_outputs, attn_W, v)
    nc = build_nc()
    res = run_bass_kernel_spmd(
        nc, in_maps, list(range(NCORES)), trace=trace, **bass_kwargs
    )
    num = np.concatenate([r["out"] for r in res.results], axis=0)    # [B, S]
    sums = np.concatenate([r["sums"] for r in res.results], axis=0)  # [B, ns]
    negm = np.concatenate([r["maxs"] for r in res.results], axis=0)  # [B, ns]
    # flash-softmax combine of the per-512-slice partials
    nb, nsl = sums.shape
    nm = -negm.astype(np.float64)
    m = nm.max(axis=1, keepdims=True)
    scale = np.exp(nm - m)                                  # [B, ns]
    num3 = num.reshape(nb, nsl, -1) * scale[:, :, None]
    tot = (sums.astype(np.float64) * scale).sum(axis=1)     # [B]
    out = num3.reshape(nb, -1) / tot[:, None]
    return out[:, None, :].astype(np.float32), res


def kernel(encoder_outputs, rnn_hidden, attn_W, attn_b, v):
    out, _ = run(encoder_outputs, rnn_hidden, attn_W, attn_b, v)
    return out



# revision 2
# speedup vs baseline: 1.7770x; 1.7770x over previous
"""Trainium2 Bass kernel for nn_AttentionModel (S=2048, B=32, H=1024).

Math: reference computes
    energy[b,s] = (enc[s,b,:] @ We.T + (h @ Wh.T + bias)) @ v  ; out = softmax_s(energy)
Since softmax is shift-invariant and the (h @ Wh.T + bias) @ v term is constant
over s, the output reduces exactly to
    out[b, 0, s] = softmax_s( enc[s,b,:] . u ),   u = v[0] @ We   (We = attn_W[:, H:])
So the kernel is a memory-bound [S*B, H] x [H] matvec + row softmax.

The matvec tolerates fp16 inputs (measured end-metric 2.4e-3 vs the 2e-2
gate; bf16 fails at 2.8e-2), so the host casts enc+u to fp16 before upload,
halving HBM traffic - the kernel's roofline.

Sharding: data-parallel over batch B across 8 cores (4 batches/core).
Device layout per core: enc [BL, jc, 128, S] fp16 (h-chunk on SBUF
partitions, s on free dim) so each h-chunk load is one fully contiguous
512 KB HBM read. PE contracts h in chunks of 128 (lhsT = u chunk [128,1],
rhs = enc tile [128,512], PSUM-accumulated); per-512-slice flash softmax
(negated max, exp with fused accumulate) on VectorE/ScalarE; numerator,
slice sums and negmaxes packed into ONE [1, 2056] row DMA'd per batch via
HWDGE on the Scalar queue. Host does the tiny 4-partial rescale + divide.
"""

import numpy as np

import concourse.bass as bass
import concourse.tile as tile
from concourse import bacc, mybir
from concourse.bass_utils import run_bass_kernel_spmd

S, B, H = 2048, 32, 1024
NCORES = 8
BL = B // NCORES  # batches per core
MM_N = 512        # matmul moving free dim (1 PSUM bank of fp32 out)
JC = H // 128     # h chunks (contraction tiles)
NS = S // MM_N    # 512-wide softmax slices per row
OUTW = S + 2 * NS  # packed output row: [exp(e) | slice sums | slice negmaxes]


def build_nc(bl=BL, s=S, enc_bufs=8, debug=False):
    """Build the per-core Bass program (SPMD: same program, different data)."""
    nc = bacc.Bacc()
    f32 = mybir.dt.float32
    f16 = mybir.dt.float16

    enc_d = nc.declare_dram_parameter("enc", [bl, JC, 128, s], f16, isOutput=False)
    u_d = nc.declare_dram_parameter("u", [128, JC], f16, isOutput=False)
    out_d = nc.declare_dram_parameter("out", [bl, OUTW], f32, isOutput=True)

    with tile.TileContext(nc) as tc:
        with (
            tc.tile_pool(name="up", bufs=1) as up,
            tc.tile_pool(name="encp", bufs=enc_bufs) as encp,
            tc.tile_pool(name="op", bufs=2) as op,
            tc.tile_pool(name="psp", bufs=2, space="PSUM") as psp,
        ):
            # Tiny u load first so the PE's weights are ready, then the
            # enc stream starts immediately behind it.
            u_sb = up.tile([128, JC], f16)
            nc.sync.dma_start(u_sb[:], u_d[:])

            for b in range(bl):
                e_ps = psp.tile([1, s], f32)
                row = op.tile([1, OUTW], f32)
                last_b = b == bl - 1
                for j in range(JC):
                    # Final chunk of the final batch streams per 512-slice
                    # so only one matmul+max+exp trail the last bytes.
                    split = NS if (last_b and j == JC - 1) else 1
                    if split == 1:
                        t = encp.tile([128, s], f16, name="t", padded_shape=[128, s])
                        nc.sync.dma_start(t[:], enc_d[b, j])
                        subs = [(0, s)]
                    else:
                        t = encp.tile([128, s], f16, name="t", padded_shape=[128, s])
                        subs = []
                        for ss in range(NS):
                            sc = slice(ss * MM_N, (ss + 1) * MM_N)
                            nc.sync.dma_start(t[:, sc], enc_d[b, j, :, sc])
                            subs.append((ss, MM_N))
                    for ss in range(NS) if split == 1 else [x[0] for x in subs]:
                        nc.tensor.matmul(
                            e_ps[:, ss * MM_N:(ss + 1) * MM_N],
                            u_sb[:, j:j + 1],
                            t[:, ss * MM_N:(ss + 1) * MM_N],
                            start=(j == 0),
                            stop=(j == JC - 1),
                        )
                        if j == JC - 1:
                            # Slice complete: negated max, then exp with
                            # fused slice-sum, overlapping remaining work.
                            nc.vector.reduce_max(
                                row[:, s + NS + ss:s + NS + ss + 1],
                                e_ps[:, ss * MM_N:(ss + 1) * MM_N],
                                axis=mybir.AxisListType.X,
                                negate=True,
                            )
                            nc.scalar.activation(
                                row[:, ss * MM_N:(ss + 1) * MM_N],
                                e_ps[:, ss * MM_N:(ss + 1) * MM_N],
                                mybir.ActivationFunctionType.Exp,
                                bias=row[:, s + NS + ss:s + NS + ss + 1],
                                accum_out=row[:, s + ss:s + ss + 1],
                            )
                # One packed HWDGE store per batch on the Scalar queue
                # (same engine as the exps -> program order, low receipt
                # latency at the tail).
                nc.scalar.dma_start(out_d[b:b + 1, :], row[:])
    nc.compile()
    return nc


def _prep_inputs(encoder_outputs, attn_W, v):
    encoder_outputs = np.asarray(encoder_outputs, dtype=np.float32)
    attn_W = np.asarray(attn_W, dtype=np.float32)
    v = np.asarray(v, dtype=np.float32)
    h = attn_W.shape[0]
    # u = v[0] @ We in float64 (host-side, tiny)
    u = (v[0].astype(np.float64) @ attn_W[:, h:].astype(np.float64)).astype(np.float16)
    u128 = np.ascontiguousarray(u.reshape(JC, 128).T)  # [128, JC] fp16
    in_maps = []
    for c in range(NCORES):
        sl = encoder_outputs[:, c * BL:(c + 1) * BL, :]
        # [BL, H, S] == [BL, JC, 128, S]: chunk loads are contiguous
        enc_c = np.ascontiguousarray(sl.transpose(1, 2, 0).astype(np.float16))
        in_maps.append({"enc": enc_c.reshape(BL, JC, 128, S), "u": u128})
    return in_maps


def run(encoder_outputs, rnn_hidden, attn_W, attn_b, v, trace=False, **bass_kwargs):
    in_maps = _prep_inputs(encoder_outputs, attn_W, v)
    nc = build_nc()
    res = run_bass_kernel_spmd(
        nc, in_maps, list(range(NCORES)), trace=trace, **bass_kwargs
    )
    packed = np.concatenate([r["out"] for r in res.results], axis=0)  # [B, OUTW]
    num = packed[:, :S]
    sums = packed[:, S:S + NS]
    negm = packed[:, S + NS:]
    # flash-softmax combine of the per-512-slice partials
    nm = -negm.astype(np.float64)
    m = nm.max(axis=1, keepdims=True)
    scale = np.exp(nm - m)                                  # [B, NS]
    num3 = num.reshape(B, NS, -1) * scale[:, :, None]
    tot = (sums.astype(np.float64) * scale).sum(axis=1)     # [B]
    out = num3.reshape(B, -1) / tot[:, None]
    return out[:, None, :].astype(np.float32), res


def kernel(encoder_outputs, rnn_hidden, attn_W, attn_b, v):
    out, _ = run(encoder_outputs, rnn_hidden, attn_W, attn_b, v)
    return out


# revision 12
# speedup vs baseline: 1.8515x; 1.0419x over previous
"""Trainium2 Bass kernel for nn_AttentionModel (S=2048, B=32, H=1024).

Math: reference computes
    energy[b,s] = (enc[s,b,:] @ We.T + (h @ Wh.T + bias)) @ v  ; out = softmax_s(energy)
Since softmax is shift-invariant and the (h @ Wh.T + bias) @ v term is constant
over s, the output reduces exactly to
    out[b, 0, s] = softmax_s( enc[s,b,:] . u ),   u = v[0] @ We   (We = attn_W[:, H:])
So the kernel is a memory-bound [S*B, H] x [H] matvec + row softmax.

The matvec tolerates fp16 inputs (measured end-metric 2.4e-3 vs the 2e-2
gate; bf16 fails at 2.8e-2), so the host casts enc+u to fp16 before upload,
halving HBM traffic - the kernel's roofline.

Sharding: data-parallel over batch B across 8 cores (4 batches/core).
Device layout per core: enc [BL, jc, 128, S] fp16 (h-chunk on SBUF
partitions, s on free dim) so each h-chunk load is one fully contiguous
512 KB HBM read. PE contracts h in chunks of 128 (lhsT = u chunk [128,1],
rhs = enc tile [128,512], PSUM-accumulated); per-512-slice flash softmax
(negated max, exp with fused accumulate) on VectorE/ScalarE; numerator,
slice sums and negmaxes packed into ONE [1, 2056] row DMA'd per batch via
HWDGE on the Scalar queue. Host does the tiny 4-partial rescale + divide.

This version is RAW Bass (no TileContext): hand-placed counting
semaphores, one stream per engine. The Tile exit path costs ~9 us of
event-semaphore clears + butterfly barriers after the last store; the raw
ending is one sync wait on the store semaphore + the block's sem-only
barrier.
"""

from contextlib import ExitStack

import numpy as np

import concourse.bass as bass
from concourse import mybir
from concourse.bass_utils import run_bass_kernel_spmd

S, B, H = 2048, 32, 1024
NCORES = 8
BL = B // NCORES   # batches per core
MM_N = 512         # matmul moving free dim (1 PSUM bank of fp32 out)
JC = H // 128      # h chunks (contraction tiles)
NS = S // MM_N     # 512-wide softmax slices per row
OUTW = S + 2 * NS  # packed output row: [exp(e) | slice sums | slice negmaxes]
NBUF = 8           # enc ring slots (one h-chunk each)


def build_nc(bl=BL, s=S):
    nc = bass.Bass(target_bir_lowering=False)
    f32 = mybir.dt.float32
    f16 = mybir.dt.float16

    enc_d = nc.dram_tensor("enc", [bl, JC, 128, s], f16, kind="ExternalInput")
    u_d = nc.dram_tensor("u", [128, JC], f16, kind="ExternalInput")
    out_d = nc.dram_tensor("out", [bl, OUTW], f32, kind="ExternalOutput")

    ctx = ExitStack()
    with ctx:
        ring = ctx.enter_context(nc.sbuf_tensor("ring", [128, NBUF * s], f16))
        u_sb = ctx.enter_context(nc.sbuf_tensor("u_sb", [128, JC], f16))
        rows = [
            ctx.enter_context(nc.sbuf_tensor(f"row{i}", [1, OUTW], f32))
            for i in range(2)
        ]
        psums = [
            ctx.enter_context(nc.psum_tensor(f"eps{i}", [1, s], f32))
            for i in range(2)
        ]
        s_u = ctx.enter_context(nc.semaphore("s_u"))        # u load done
        # One DMA-completion sem per ring slot: a single counting sem is
        # unsound (completions of different DMAs may inc out of order);
        # per-slot sems + an issue-side same-sem wait before reuse give
        # the in-order guarantee (the Tile DMAHW-lane pattern).
        s_enc = [
            ctx.enter_context(nc.semaphore(f"s_enc{q}")) for q in range(NBUF)
        ]
        s_mm = ctx.enter_context(nc.semaphore("s_mm"))      # j<7 chunks consumed
        s_slice = ctx.enter_context(nc.semaphore("s_slice"))  # slice-final matmuls
        s_max = ctx.enter_context(nc.semaphore("s_max"))    # negmaxes written
        s_exp = ctx.enter_context(nc.semaphore("s_exp"))    # exps done (PSUM free)
        s_row = [
            ctx.enter_context(nc.semaphore(f"s_row{i}")) for i in range(2)
        ]                                                   # row stores done
        s_tail = [
            ctx.enter_context(nc.semaphore(f"s_tail{i}")) for i in range(NS)
        ]                                                   # final-chunk slices

        def slot(k):
            sl = k % NBUF
            return ring[:, sl * s:(sl + 1) * s]

        with nc.Block(no_gpsimd_drain=True) as block:

            @block.sync
            def _(sync):
                sync.dma_start(u_sb[:, :], u_d[:, :]).then_inc(s_u, 16)
                for b in range(bl):
                    for j in range(JC):
                        k = b * JC + j
                        q = k % NBUF
                        n = k // NBUF  # prior uses of this slot
                        tail = b == bl - 1 and j == JC - 1
                        # ring slot free once chunk k-NBUF is consumed;
                        # the same-sem wait also orders this DMA's sem
                        # update after the previous one on this slot.
                        if k >= NBUF:
                            sync.wait_ge(s_enc[q], 16 * n)
                            if j < JC - 1:
                                sync.wait_ge(s_mm, (b - 1) * (JC - 1) + j + 1)
                            else:
                                sync.wait_ge(s_slice, NS * b)
                        if not tail:
                            sync.dma_start(slot(k), enc_d[b, j]).then_inc(
                                s_enc[q], 16
                            )
                        else:
                            # stream the final chunk per 512-slice so only
                            # one matmul+max+exp trail the last bytes
                            for ss in range(NS):
                                sc = slice(ss * MM_N, (ss + 1) * MM_N)
                                sync.dma_start(
                                    slot(k)[:, sc], enc_d[b, j, :, sc]
                                ).then_inc(s_tail[ss], 16)
                # all stores landed before the NEFF retires
                sync.wait_ge(s_row[0], 16 * (bl // 2))
                sync.wait_ge(s_row[1], 16 * (bl - bl // 2))

            @block.tensor
            def _(tensor):
                tensor.wait_ge(s_u, 16)
                for b in range(bl):
                    e_ps = psums[b % 2]
                    for j in range(JC):
                        k = b * JC + j
                        q = k % NBUF
                        n = k // NBUF
                        tail = b == bl - 1 and j == JC - 1
                        if j == 0 and b >= 2:
                            # PSUM buffer free once batch b-2's exps read it
                            tensor.wait_ge(s_exp, NS * (b - 1))
                        if not tail:
                            tensor.wait_ge(s_enc[q], 16 * (n + 1))
                        for ss in range(NS):
                            if tail:
                                tensor.wait_ge(s_tail[ss], 16)
                            mm = tensor.matmul(
                                e_ps[:, ss * MM_N:(ss + 1) * MM_N],
                                u_sb[:, j:j + 1],
                                slot(k)[:, ss * MM_N:(ss + 1) * MM_N],
                                start=(j == 0),
                                stop=(j == JC - 1),
                            )
                            if j == JC - 1:
                                mm.then_inc(s_slice, 1)
                            elif ss == NS - 1:
                                mm.then_inc(s_mm, 1)

            @block.vector
            def _(vector):
                for b in range(bl):
                    row = rows[b % 2]
                    for ss in range(NS):
                        if ss == 0 and b >= 2:
                            # row buffer free once batch b-2's store landed
                            vector.wait_ge(s_row[b % 2], 16 * (b // 2))
                        vector.wait_ge(s_slice, NS * b + ss + 1)
                        vector.reduce_max(
                            row[:, S + NS + ss:S + NS + ss + 1],
                            psums[b % 2][:, ss * MM_N:(ss + 1) * MM_N],
                            axis=mybir.AxisListType.X,
                            negate=True,
                        ).then_inc(s_max, 1)

            @block.scalar
            def _(scalar):
                for b in range(bl):
                    row = rows[b % 2]
                    for ss in range(NS):
                        if ss == 0 and b >= 2:
                            # row buffer free once batch b-2's store landed
                            # (also orders this batch's store-sem update)
                            scalar.wait_ge(s_row[b % 2], 16 * (b // 2))
                        scalar.wait_ge(s_max, NS * b + ss + 1)
                        scalar.activation(
                            row[:, ss * MM_N:(ss + 1) * MM_N],
                            psums[b % 2][:, ss * MM_N:(ss + 1) * MM_N],
                            mybir.ActivationFunctionType.Exp,
                            bias=row[:, S + NS + ss:S + NS + ss + 1],
                            accum_out=row[:, S + ss:S + ss + 1],
                        ).then_inc(s_exp, 1)
                    # the store's SBUF read is asynchronous: it must wait on
                    # the exps' sem even though they issued earlier here
                    scalar.wait_ge(s_exp, NS * (b + 1))
                    scalar.dma_start(out_d[b:b + 1, :], row[:, :]).then_inc(
                        s_row[b % 2], 16
                    )

            @block.gpsimd
            def _(gpsimd):
                pass

    return nc


def _prep_inputs(encoder_outputs, attn_W, v):
    encoder_outputs = np.asarray(encoder_outputs, dtype=np.float32)
    attn_W = np.asarray(attn_W, dtype=np.float32)
    v = np.asarray(v, dtype=np.float32)
    h = attn_W.shape[0]
    # u = v[0] @ We in float64 (host-side, tiny)
    u = (v[0].astype(np.float64) @ attn_W[:, h:].astype(np.float64)).astype(np.float16)
    u128 = np.ascontiguousarray(u.reshape(JC, 128).T)  # [128, JC] fp16
    in_maps = []
    for c in range(NCORES):
        sl = encoder_outputs[:, c * BL:(c + 1) * BL, :]
        # [BL, H, S] == [BL, JC, 128, S]: chunk loads are contiguous
        enc_c = np.ascontiguousarray(sl.transpose(1, 2, 0).astype(np.float16))
        in_maps.append({"enc": enc_c.reshape(BL, JC, 128, S), "u": u128})
    return in_maps


def run(encoder_outputs, rnn_hidden, attn_W, attn_b, v, trace=False, **bass_kwargs):
    in_maps = _prep_inputs(encoder_outputs, attn_W, v)
    nc = build_nc()
    res = run_bass_kernel_spmd(
        nc, in_maps, list(range(NCORES)), trace=trace, **bass_kwargs
    )
    packed = np.concatenate([r["out"] for r in res.results], axis=0)  # [B, OUTW]
    num = packed[:, :S]
    sums = packed[:, S:S + NS]
    negm = packed[:, S + NS:]
    # flash-softmax combine of the per-512-slice partials
    nm = -negm.astype(np.float64)
    m = nm.max(axis=1, keepdims=True)
    scale = np.exp(nm - m)                                  # [B, NS]
    num3 = num.reshape(B, NS, -1) * scale[:, :, None]
    tot = (sums.astype(np.float64) * scale).sum(axis=1)     # [B]
    out = num3.reshape(B, -1) / tot[:, None]
    return out[:, None, :].astype(np.float32), res


def kernel(encoder_outputs, rnn_hidden, attn_W, attn_b, v):
    out, _ = run(encoder_outputs, rnn_hidden, attn_W, attn_b, v)
    return out


# revision 13
# speedup vs baseline: 1.9424x; 1.0490x over previous
"""Trainium2 Bass kernel for nn_AttentionModel (S=2048, B=32, H=1024).

Math: reference computes
    energy[b,s] = (enc[s,b,:] @ We.T + (h @ Wh.T + bias)) @ v  ; out = softmax_s(energy)
Since softmax is shift-invariant and the (h @ Wh.T + bias) @ v term is constant
over s, the output reduces exactly to
    out[b, 0, s] = softmax_s( enc[s,b,:] . u ),   u = v[0] @ We   (We = attn_W[:, H:])
So the kernel is a memory-bound [S*B, H] x [H] matvec + row softmax.

The matvec tolerates fp16 inputs (measured end-metric 2.4e-3 vs the 2e-2
gate; bf16 fails at 2.8e-2), so the host casts enc+u to fp16 before upload,
halving HBM traffic - the kernel's roofline.

Sharding: data-parallel over batch B across 8 cores (4 batches/core).
Device layout per core: enc [BL, jc, 128, NS, 512] fp16 (h-chunk on SBUF
partitions, s on free dim) so each h-chunk load is one fully contiguous
512 KB HBM read. PE contracts h in chunks of 128 (lhsT = u chunk [128,1],
rhs = enc tile [128,512], PSUM-accumulated); per-512-slice flash softmax
(negated max, exp with fused slice-sum) on VectorE/ScalarE; numerator,
slice sums and negmaxes packed into ONE [1, 2056] row DMA'd per batch via
HWDGE on the Scalar queue. Host does the tiny 4-partial rescale + divide.

Tail shaping: the last batch's exps (2048 elems on ONE ScalarE lane,
~2.7 us serial) must not all land after the final bytes. Its chunks 5-7
are re-laid host-side into enc_tail [NS, 128, 3, 512] (slice-major,
contiguous 384 KB per slice) and streamed slice-by-slice, so each slice's
final matmul -> max -> exp runs ~1.1 us apart while the next slice still
streams; only slice 3's chain plus a small 520-element store trail the
last HBM byte.
"""

import numpy as np

import concourse.bass as bass
import concourse.tile as tile
from concourse import bacc, mybir
from concourse.bass_utils import run_bass_kernel_spmd

S, B, H = 2048, 32, 1024
NCORES = 8
BL = B // NCORES   # batches per core
MM_N = 512         # matmul moving free dim (1 PSUM bank of fp32 out)
JC = H // 128      # h chunks (contraction tiles)
NS = S // MM_N     # 512-wide softmax slices per row
OUTW = S + 2 * NS  # packed output row: [exp(e) | slice sums | slice negmaxes]
JTAIL = 5          # last batch: chunks [JTAIL..JC) stream slice-major


def build_nc(bl=BL, enc_bufs=8):
    """Build the per-core Bass program (SPMD: same program, different data)."""
    nc = bacc.Bacc()
    f32 = mybir.dt.float32
    f16 = mybir.dt.float16
    ntc = JC - JTAIL  # tail chunks per slice

    enc_d = nc.declare_dram_parameter(
        "enc", [bl, JC, 128, NS, MM_N], f16, isOutput=False
    )
    tail_d = nc.declare_dram_parameter(
        "enc_tail", [NS, 128, ntc, MM_N], f16, isOutput=False
    )
    u_d = nc.declare_dram_parameter("u", [128, JC], f16, isOutput=False)
    out_d = nc.declare_dram_parameter("out", [bl, OUTW], f32, isOutput=True)

    with tile.TileContext(nc) as tc:
        with (
            tc.tile_pool(name="up", bufs=1) as up,
            tc.tile_pool(name="encp", bufs=enc_bufs) as encp,
            tc.tile_pool(name="op", bufs=2) as op,
            tc.tile_pool(name="psp", bufs=2, space="PSUM") as psp,
        ):
            # First enc chunk starts the stream immediately; tiny u load
            # rides right behind it (first matmul is ~1.5 us out anyway).
            t0 = encp.tile([128, NS, MM_N], f16, name="t",
                           padded_shape=[128, NS, MM_N])
            nc.sync.dma_start(t0[:], enc_d[0, 0])
            u_sb = up.tile([128, JC], f16)
            nc.sync.dma_start(u_sb[:], u_d[:])

            def mm(e_ps, j, ss, rhs):
                nc.tensor.matmul(
                    e_ps[:, ss * MM_N:(ss + 1) * MM_N],
                    u_sb[:, j:j + 1],
                    rhs,
                    start=(j == 0),
                    stop=(j == JC - 1),
                )

            def softmax_slice(e_ps, row, ss):
                nc.vector.reduce_max(
                    row[:, S + NS + ss:S + NS + ss + 1],
                    e_ps[:, ss * MM_N:(ss + 1) * MM_N],
                    axis=mybir.AxisListType.X,
                    negate=True,
                )
                nc.scalar.activation(
                    row[:, ss * MM_N:(ss + 1) * MM_N],
                    e_ps[:, ss * MM_N:(ss + 1) * MM_N],
                    mybir.ActivationFunctionType.Exp,
                    bias=row[:, S + NS + ss:S + NS + ss + 1],
                    accum_out=row[:, S + ss:S + ss + 1],
                )

            for b in range(bl):
                e_ps = psp.tile([1, S], f32)
                row = op.tile([1, OUTW], f32)
                last_b = b == bl - 1
                jmax = JTAIL if last_b else JC
                for j in range(jmax):
                    if b == 0 and j == 0:
                        t = t0
                    else:
                        t = encp.tile([128, NS, MM_N], f16, name="t",
                                      padded_shape=[128, NS, MM_N])
                        nc.sync.dma_start(t[:], enc_d[b, j])
                    for ss in range(NS):
                        mm(e_ps, j, ss, t[:, ss, :])
                        if j == JC - 1:
                            softmax_slice(e_ps, row, ss)
                if not last_b:
                    nc.scalar.dma_start(out_d[b:b + 1, :], row[:])
                else:
                    # slice-major tail: each slice completes while the
                    # next one still streams
                    for ss in range(NS):
                        tt = encp.tile([128, ntc, MM_N], f16, name="t",
                                       padded_shape=[128, NS, MM_N])
                        nc.sync.dma_start(tt[:], tail_d[ss])
                        for c in range(ntc):
                            mm(e_ps, JTAIL + c, ss, tt[:, c, :])
                        softmax_slice(e_ps, row, ss)
                        if ss == NS - 2:
                            # everything but slice 3's numerator is ready:
                            # push the bulk of the row out early
                            nc.scalar.dma_start(
                                out_d[b:b + 1, :(NS - 1) * MM_N],
                                row[:, :(NS - 1) * MM_N],
                            )
                    nc.scalar.dma_start(
                        out_d[b:b + 1, (NS - 1) * MM_N:],
                        row[:, (NS - 1) * MM_N:],
                    )
    nc.compile()
    return nc


def _prep_inputs(encoder_outputs, attn_W, v):
    encoder_outputs = np.asarray(encoder_outputs, dtype=np.float32)
    attn_W = np.asarray(attn_W, dtype=np.float32)
    v = np.asarray(v, dtype=np.float32)
    h = attn_W.shape[0]
    ntc = JC - JTAIL
    # u = v[0] @ We in float64 (host-side, tiny)
    u = (v[0].astype(np.float64) @ attn_W[:, h:].astype(np.float64)).astype(np.float16)
    u128 = np.ascontiguousarray(u.reshape(JC, 128).T)  # [128, JC] fp16
    in_maps = []
    for c in range(NCORES):
        sl = encoder_outputs[:, c * BL:(c + 1) * BL, :]
        # [BL, H, S] == [BL, JC, 128, NS, MM_N]: chunk loads are contiguous
        enc_c = np.ascontiguousarray(sl.transpose(1, 2, 0).astype(np.float16))
        enc_c = enc_c.reshape(BL, JC, 128, NS, MM_N)
        # last batch's tail chunks, slice-major so each slice is one
        # contiguous 384 KB read
        tail = np.ascontiguousarray(
            enc_c[BL - 1, JTAIL:].transpose(2, 1, 0, 3)
        )  # [NS, 128, ntc, MM_N]
        in_maps.append({"enc": enc_c, "enc_tail": tail, "u": u128})
    return in_maps


def run(encoder_outputs, rnn_hidden, attn_W, attn_b, v, trace=False, **bass_kwargs):
    in_maps = _prep_inputs(encoder_outputs, attn_W, v)
    nc = build_nc()
    res = run_bass_kernel_spmd(
        nc, in_maps, list(range(NCORES)), trace=trace, **bass_kwargs
    )
    packed = np.concatenate([r["out"] for r in res.results], axis=0)  # [B, OUTW]
    num = packed[:, :S]
    sums = packed[:, S:S + NS]
    negm = packed[:, S + NS:]
    # flash-softmax combine of the per-512-slice partials
    nm = -negm.astype(np.float64)
    m = nm.max(axis=1, keepdims=True)
    scale = np.exp(nm - m)                                  # [B, NS]
    num3 = num.reshape(B, NS, -1) * scale[:, :, None]
    tot = (sums.astype(np.float64) * scale).sum(axis=1)     # [B]
    out = num3.reshape(B, -1) / tot[:, None]
    return out[:, None, :].astype(np.float32), res


def kernel(encoder_outputs, rnn_hidden, attn_W, attn_b, v):
    out, _ = run(encoder_outputs, rnn_hidden, attn_W, attn_b, v)
    return out
